# revision 25
# baseline (speedup 1.0000x reference)
"""Causal multi-head attention (dense transformer block) on 8 Trainium2 cores.

Problem: x[4, 2048, 1024], 16 heads, head_dim 64, causal softmax attention
with QKV + output projections (torch Linear layout weights).

Sharding: 8 cores = 4 batches x 2 head-groups (8 heads each).  Each core
computes QKV projection for its 8 heads, attention, and its partial output
projection (row-parallel over w_out).  Host sums the two partials per batch
and adds b_out.

All device layouts are "transposed" so no on-device transposes are needed:
  - x is fed as xT [d, s]; Q^T/K^T are produced as [head_dim, s]
  - scores are computed as S^T [k, q]; softmax runs along partitions via
    ones-matmul column sums; O is accumulated directly as O^T [e_loc, q],
    which is exactly the lhsT the output projection needs.
Matmul inputs are bf16 (PSUM accumulation is fp32); everything else fp32.

Schedule (v2): the whole kernel is software-pipelined around the ACT
engine's exp stream, which is the irreducible resource (~151us):
  - head: V-projection starts inside the input-DMA window (kt-outer over 6
    PSUM banks), then only K(hp0,hp1) + Q(hp0,hp1;chunk3) run before
    attention begins (~37us to first exp instead of ~78us).
  - attention: flat iteration stream over units (j,pair), pair 0 units
    first.  Scores for iteration t+1 issue ahead of PV(t); the 4
    denominator ones-matmuls issue after both PV pairs so all four run
    tile-concurrently on disjoint 32-col groups (one ~213ns slot).
    Between the two half-periods a "fill" slot issues deferred work:
    V pass 2, remaining Q/K projection chains, output-projection groups,
    and softmax-denominator broadcast/normalize chains.
  - PV PSUM accumulators are copied to SBUF (raw, bf16) immediately at
    unit end so the PSUM banks recycle without waiting on the normalize
    chain; normalization multiplies happen later as deferred DVE work.
  - tail: only chunk 0's output projection, on a 4-buffer PSUM pool.
"""

import sys

sys.path.insert(0, "/opt/trn_rl_repo")

import numpy as np
import ml_dtypes

import concourse.bass as bass
import concourse.mybir as mybir
import concourse.tile as tile
from concourse import bacc
from concourse import bass_utils
from concourse import library_config
from concourse.masks import make_upper_triangular

F32 = mybir.dt.float32
BF16 = mybir.dt.bfloat16
EXP = mybir.ActivationFunctionType.Exp

B, S, D = 4, 2048, 1024
HTOT, HD = 16, 64
NCORES = 8
HLOC = HTOT // 2          # heads per core
ELOC = HLOC * HD          # 512 local embedding width
NHP = HLOC // 2           # 4 head pairs
QC = 512                  # q-chunk width
NQC = S // QC             # 4
NKT = S // 128            # 16 k tiles over sequence
NDT = D // 128            # 8 k tiles over model dim
SCALE = 1.0 / float(np.sqrt(HD))

_CACHE = {}


def _build_nc():
    nc = bacc.Bacc("TRN2", target_bir_lowering=False, debug=False)

    xT = nc.dram_tensor("xT", [D, S], BF16, kind="ExternalInput")
    wqT = nc.dram_tensor("wqT", [D, ELOC], BF16, kind="ExternalInput")
    wkT = nc.dram_tensor("wkT", [D, ELOC], BF16, kind="ExternalInput")
    wvT = nc.dram_tensor("wvT", [D, ELOC], BF16, kind="ExternalInput")
    woT = nc.dram_tensor("woT", [ELOC, D], BF16, kind="ExternalInput")
    bqk = nc.dram_tensor("bqk", [128, 2, NHP], F32, kind="ExternalInput")
    bvb = nc.dram_tensor("bvb", [128, ELOC], F32, kind="ExternalInput")
    outp = nc.dram_tensor("outp", [S, D], F32, kind="ExternalOutput")

    with tile.TileContext(nc) as tc:
        with tc.tile_pool(name="const", bufs=1) as constp, \
             tc.tile_pool(name="wpool", bufs=1) as wp, \
             tc.tile_pool(name="qkv", bufs=1) as qkvp, \
             tc.tile_pool(name="xt", bufs=1) as xtp, \
             tc.tile_pool(name="pt", bufs=8) as ptp, \
             tc.tile_pool(name="otn", bufs=16) as otnp, \
             tc.tile_pool(name="our", bufs=12) as ourp, \
             tc.tile_pool(name="dr", bufs=4) as drp, \
             tc.tile_pool(name="osb", bufs=6) as osbp:

            # ---- constants ----
            # wsrc feeds the PE warm-up matmuls; memset on DVE so the PE can
            # start at ~0 instead of queueing behind gpsimd const work.
            wsrc = constp.tile([128, ELOC], BF16, name="wsrc")
            nc.vector.memset(wsrc[:], 0.0)
            trimask = constp.tile([128, 128], BF16, name="trimask")
            make_upper_triangular(nc, trimask[:], val=1.0, diag=True)
            ones_a = constp.tile([128, 1], BF16, name="ones_a")
            nc.gpsimd.memset(ones_a[:], 1.0)
            ones_b = constp.tile([128, 2], BF16, name="ones_b")
            nc.gpsimd.memset(ones_b[:], 0.0)
            nc.gpsimd.memset(ones_b[:, 0:1], 1.0)
            bc_x = constp.tile([98, 128], BF16, name="bc_x")
            nc.gpsimd.memset(bc_x[:], 0.0)
            nc.gpsimd.memset(bc_x[0:1, 0:64], 1.0)
            nc.gpsimd.memset(bc_x[32:33, 64:128], 1.0)
            bc_y = constp.tile([98, 128], BF16, name="bc_y")
            nc.gpsimd.memset(bc_y[:], 0.0)
            nc.gpsimd.memset(bc_y[64:65, 0:64], 1.0)
            nc.gpsimd.memset(bc_y[96:97, 64:128], 1.0)

            bqk_sb = constp.tile([128, 2, NHP], F32, name="bqk_sb")
            nc.sync.dma_start(bqk_sb[:], bqk[:])
            bvb_sb = constp.tile([128, ELOC], F32, name="bvb_sb")
            nc.sync.dma_start(bvb_sb[:], bvb[:])

            # ---- input DMAs: V-projection inputs first, interleaved so
            # the kt-outer V pass can start as soon as (wv0, xt0) land.
            # wq/wk/wo issue from other sequencers to parallelize DGE setup.
            wv_sb, xts = [], []
            for kt in range(NDT):
                t = wp.tile([128, ELOC], BF16, name=f"wv{kt}")
                nc.sync.dma_start(t[:], wvT[128 * kt:128 * (kt + 1), :])
                wv_sb.append(t)
                t = xtp.tile([128, S], BF16, name=f"xt{kt}")
                nc.sync.dma_start(t[:], xT[128 * kt:128 * (kt + 1), :])
                xts.append(t)
            wq_sb, wk_sb = [], []
            for kt in range(NDT):
                for lst, srct, nm in ((wq_sb, wqT, "wq"), (wk_sb, wkT, "wk")):
                    t = wp.tile([128, ELOC], BF16, name=f"{nm}{kt}")
                    nc.scalar.dma_start(t[:], srct[128 * kt:128 * (kt + 1), :])
                    lst.append(t)
            wo_sb = []
            for hp in range(NHP):
                t = wp.tile([128, D], BF16, name=f"wo{hp}")
                nc.scalar.dma_start(t[:], woT[128 * hp:128 * (hp + 1), :])
                wo_sb.append(t)

            # ---- QKV destination tiles ----
            QT, KT = [], []
            for hp in range(NHP):
                QT.append(qkvp.tile([128, S], BF16, name=f"qt{hp}"))
                KT.append(qkvp.tile([128, S], BF16, name=f"kt{hp}"))
            V = [qkvp.tile([128, ELOC], BF16, name=f"v{st}") for st in range(NKT)]

            def v_chain(pool, tag, st):
                ps = pool.tile([128, ELOC], F32, tag=tag, name="ps_f")
                for kt in range(NDT):
                    nc.tensor.matmul(
                        ps[:],
                        lhsT=xts[kt][:, 128 * st:128 * (st + 1)],
                        rhs=wv_sb[kt][:],
                        start=(kt == 0), stop=(kt == NDT - 1))
                nc.vector.tensor_add(V[st][:], ps[:], bvb_sb[:])

            def qk_chain(pool, tag, qk, hp, c):
                dst = (QT if qk == 0 else KT)[hp]
                wsb = wq_sb if qk == 0 else wk_sb
                ps = pool.tile([128, QC], F32, tag=tag, name="ps_f")
                for kt in range(NDT):
                    nc.tensor.matmul(
                        ps[:],
                        lhsT=wsb[kt][:, 128 * hp:128 * (hp + 1)],
                        rhs=xts[kt][:, QC * c:QC * (c + 1)],
                        start=(kt == 0), stop=(kt == NDT - 1))
                nc.vector.tensor_scalar_add(
                    dst[:, QC * c:QC * (c + 1)], ps[:],
                    bqk_sb[:, qk, hp:hp + 1])

            # ---- head: V pass 1 (st 0..5 kt-outer inside the DMA window,
            # then st 6,7), K(0), K(1) all chunks, Q(0,c3), Q(1,c3). ----
            with tc.tile_pool(name="psq", bufs=8, space="PSUM") as psq:
                # PE warm-up at t~0: keeps the HAM activity monitor busy so
                # the first real matmuls run at 2.4 GHz.  Depends only on
                # the DVE wsrc memset, not the DMAs or gpsimd consts.
                wups = psq.tile([128, ELOC], F32, tag="psq", name="wups")
                for r in range(8):
                    nc.tensor.matmul(wups[:], lhsT=wsrc[:, 0:128],
                                     rhs=wsrc[:],
                                     start=(r == 0), stop=(r == 7))
                # mega kt-loop using all 8 PSUM banks: V st0-3 and K(0)'s
                # four chunks accumulate together, so the DMA window is
                # filled with ~8 matmuls per arriving (wv,xt,wk) tile set.
                psv = [psq.tile([128, ELOC], F32, tag="psq", name=f"psv{st}")
                       for st in range(4)]
                psk = [psq.tile([128, QC], F32, tag="psq", name=f"psk{c}")
                       for c in range(NQC)]
                for kt in range(NDT - 1):
                    for st in range(4):
                        nc.tensor.matmul(
                            psv[st][:],
                            lhsT=xts[kt][:, 128 * st:128 * (st + 1)],
                            rhs=wv_sb[kt][:],
                            start=(kt == 0), stop=False)
                    for c in range(NQC):
                        nc.tensor.matmul(
                            psk[c][:],
                            lhsT=wk_sb[kt][:, 0:128],
                            rhs=xts[kt][:, QC * c:QC * (c + 1)],
                            start=(kt == 0), stop=False)
                # peel kt=7 so each PSUM drain (DVE add) pipelines behind
                # the next accumulator's matmul instead of serializing at
                # the end (which idled the PE enough to re-throttle HAM).
                for st in range(4):
                    nc.tensor.matmul(
                        psv[st][:],
                        lhsT=xts[NDT - 1][:, 128 * st:128 * (st + 1)],
                        rhs=wv_sb[NDT - 1][:],
                        start=False, stop=True)
                    nc.vector.tensor_add(V[st][:], psv[st][:], bvb_sb[:])
                for c in range(NQC):
                    nc.tensor.matmul(
                        psk[c][:],
                        lhsT=wk_sb[NDT - 1][:, 0:128],
                        rhs=xts[NDT - 1][:, QC * c:QC * (c + 1)],
                        start=False, stop=True)
                    nc.vector.tensor_scalar_add(
                        KT[0][:, QC * c:QC * (c + 1)], psk[c][:],
                        bqk_sb[:, 1, 0:1])
                for st in (4, 5, 6, 7):
                    v_chain(psq, "psq", st)
                # K(1) kt-outer over 4 chunk-banks (shared stationary wk
                # slice across the inner matmuls).
                psl = [psq.tile([128, QC], F32, tag="psq", name=f"psl{c}")
                       for c in range(NQC)]
                for kt in range(NDT - 1):
                    for c in range(NQC):
                        nc.tensor.matmul(
                            psl[c][:],
                            lhsT=wk_sb[kt][:, 128:256],
                            rhs=xts[kt][:, QC * c:QC * (c + 1)],
                            start=(kt == 0), stop=False)
                for c in range(NQC):
                    nc.tensor.matmul(
                        psl[c][:],
                        lhsT=wk_sb[NDT - 1][:, 128:256],
                        rhs=xts[NDT - 1][:, QC * c:QC * (c + 1)],
                        start=False, stop=True)
                    nc.vector.tensor_scalar_add(
                        KT[1][:, QC * c:QC * (c + 1)], psl[c][:],
                        bqk_sb[:, 1, 1:2])
                qk_chain(psq, "psq", 0, 0, 3)
                qk_chain(psq, "psq", 0, 1, 3)

            # ---- attention: flat pipelined stream over units ----
            # units: pair 0 first so K/Q for hps 2,3 and all output
            # projections can be deferred into fill slots.  Within pair 1,
            # small chunks run first so each chunk's output projection has
            # enough later fill slots; the big (3,1) unit is last and its
            # output projection runs in the tail.
            units = [(3, 0), (2, 0), (1, 0), (0, 0),
                     (1, 1), (0, 1), (2, 1), (3, 1)]
            iters = []
            for u, (j, pair) in enumerate(units):
                nkt = 4 * j + 4
                for i in range(nkt):
                    iters.append((u, j, pair, i, nkt))
            NIT = len(iters)

            tri3 = trimask[:][:, None, :].broadcast_to([128, 2, 128])

            # deferred-work queue; one item issued per iteration between the
            # two half-periods.  Items are closures issuing ~0.2-1.8us of PE
            # work (plus DVE followups).
            fills = []
            # V pass 2 then the Q/K chains needed before their deadlines:
            # Q(0/1,c2) by unit (2,0); K2/K3 and Q(2/3,c3) by unit (3,1);
            # Q(0/1,c1) by (1,0); Q(0/1,c0) by (0,0); Q(2/3,c2..c0) by the
            # matching pair-1 units.  Pops: item k at global iteration k.
            for st in range(8, NKT):
                fills.append(lambda st=st: v_chain(psop, "ps", st))
            for hp in (0, 1):
                fills.append(lambda hp=hp: qk_chain(psop, "ps", 0, hp, 2))
            for c in range(NQC):
                fills.append(lambda c=c: qk_chain(psop, "ps", 1, 2, c))
            for c in range(NQC):
                fills.append(lambda c=c: qk_chain(psop, "ps", 1, 3, c))
            for hp in (0, 1):
                fills.append(lambda hp=hp: qk_chain(psop, "ps", 0, hp, 1))
            for hp in (0, 1):
                fills.append(lambda hp=hp: qk_chain(psop, "ps", 0, hp, 0))
            fills.append(lambda: qk_chain(psop, "ps", 0, 2, 3))
            fills.append(lambda: qk_chain(psop, "ps", 0, 3, 3))
            for c in (2, 1, 0):
                fills.append(lambda c=c: qk_chain(psop, "ps", 0, 2, c))
                fills.append(lambda c=c: qk_chain(psop, "ps", 0, 3, c))

            otn_map = {}   # (j, hp) -> normalized O^T tile (bf16, SBUF)

            def outproj_group(j, m, eo):
                s0 = QC * j + 128 * m
                ps_o = psop.tile([128, 512], F32, tag="ps", name="ps_f")
                for hp in range(NHP):
                    nc.tensor.matmul(
                        ps_o[:],
                        lhsT=otn_map[(j, hp)][:, 128 * m:128 * (m + 1)],
                        rhs=wo_sb[hp][:, 512 * eo:512 * (eo + 1)],
                        start=(hp == 0), stop=(hp == NHP - 1))
                osb = osbp.tile([128, 512], F32)
                nc.vector.tensor_copy(osb[:], ps_o[:])
                nc.sync.dma_start(
                    outp[s0:s0 + 128, 512 * eo:512 * (eo + 1)], osb[:])

            def norm_chain(dr, raw, j, hp, ih):
                # broadcast the two denominator-reciprocal rows for this hp
                # to 128 partitions via PE, then normalize the raw PV copy.
                bc = bc_x if ih == 0 else bc_y
                ps_b = psop.tile([128, QC], F32, tag="ps", name="ps_f")
                nc.tensor.matmul(ps_b[:], lhsT=bc[:], rhs=dr[:],
                                 start=True, stop=True)
                dbc = drp.tile([128, QC], BF16, name="dbc")
                nc.vector.tensor_copy(dbc[:], ps_b[:])
                otn = otnp.tile([128, QC], BF16, tag="otn", name="otn")
                nc.vector.tensor_mul(otn[:], raw[:], dbc[:])
                otn_map[(j, hp)] = otn

            TAIL_J = units[-1][0]

            with tc.tile_pool(name="pss", bufs=2, space="PSUM") as pss, \
                 tc.tile_pool(name="psov", bufs=2, space="PSUM") as psov, \
                 tc.tile_pool(name="psd", bufs=1, space="PSUM") as psd, \
                 tc.tile_pool(name="psop", bufs=1, space="PSUM") as psop:

                def issue_scores(t, ih):
                    u, j, pair, i, nkt = iters[t]
                    hp = 2 * pair + ih
                    w = 128 * (i - 4 * j) if i >= 4 * j else 0
                    ps_s = pss.tile([128, 2, QC], F32, tag="pss",
                                    name="ps_s")
                    for h2 in range(2):
                        nc.tensor.matmul(
                            ps_s[:, h2, w:QC],
                            lhsT=KT[hp][64 * h2:64 * (h2 + 1),
                                        128 * i:128 * (i + 1)],
                            rhs=QT[hp][64 * h2:64 * (h2 + 1),
                                       QC * j + w:QC * (j + 1)],
                            start=True, stop=True)
                    return ps_s

                # prologue: scores for iteration 0
                sc_next = [issue_scores(0, 0), issue_scores(0, 1)]
                cur_ot = None   # [ps_ot_a, ps_ot_b] accumulators of the unit
                ps_d = None

                for t in range(NIT):
                    u, j, pair, i, nkt = iters[t]
                    hps = (2 * pair, 2 * pair + 1)
                    w = 128 * (i - 4 * j) if i >= 4 * j else 0
                    last = (i == nkt - 1)
                    ps_pair = sc_next
                    sc_next = [None, None]

                    if i == 0:
                        cur_ot = [psov.tile([128, QC], F32, tag="psov",
                                            name=f"ps_ot{hp}")
                                  for hp in hps]
                        ps_d = psd.tile([128, QC], F32, tag="psd",
                                        name="ps_d")

                    # exp for both hps (ACT stream; masks on DVE)
                    pts = []
                    for ih, hp in enumerate(hps):
                        pt = ptp.tile([128, 2, QC], BF16, tag="pt",
                                      name="pt")
                        nc.scalar.activation(pt[:, :, w:QC],
                                             ps_pair[ih][:, :, w:QC],
                                             EXP, scale=SCALE)
                        if i >= 4 * j:
                            nc.vector.tensor_mul(
                                pt[:, :, w:w + 128],
                                pt[:, :, w:w + 128], tri3[:, :, :])
                        pts.append(pt)

                    def issue_pv(ih, hp):
                        for h2 in range(2):
                            nc.tensor.matmul(
                                cur_ot[ih][64 * h2:64 * (h2 + 1), w:QC],
                                lhsT=V[i][:, 64 * (2 * hp + h2):
                                          64 * (2 * hp + h2 + 1)],
                                rhs=pts[ih][:, h2, w:QC],
                                start=(i == 0), stop=last,
                                tile_position=(0, 64 * h2))

                    # PE order: lead with the instruction whose gate opens
                    # first (PV(t,a) on exp(t,a); scores lead on masked
                    # boundary tiles since PV also waits for the DVE mask),
                    # then keep the two score pairs adjacent — same-kind
                    # matmuls stream back-to-back with no weight-load
                    # exposure, unlike kind-switches (~+100ns each).
                    bdry = i >= 4 * j
                    if bdry:
                        if t + 1 < NIT:
                            sc_next[0] = issue_scores(t + 1, 0)
                            sc_next[1] = issue_scores(t + 1, 1)
                        issue_pv(0, hps[0])
                        issue_pv(1, hps[1])
                    else:
                        issue_pv(0, hps[0])
                        if t + 1 < NIT:
                            sc_next[0] = issue_scores(t + 1, 0)
                            sc_next[1] = issue_scores(t + 1, 1)
                        issue_pv(1, hps[1])

                    for ih in range(2):
                        nc.tensor.matmul(
                            ps_d[64 * ih:64 * ih + 1, w:QC],
                            lhsT=ones_a[:], rhs=pts[ih][:, 0, w:QC],
                            start=(i == 0), stop=last,
                            tile_position=(0, 64 * ih))
                        nc.tensor.matmul(
                            ps_d[64 * ih + 32:64 * ih + 34, w:QC],
                            lhsT=ones_b[:], rhs=pts[ih][:, 1, w:QC],
                            start=(i == 0), stop=last,
                            tile_position=(0, 64 * ih + 32))

                    if fills:
                        fills.pop(0)()

                    if last:
                        # unit end: denominator reciprocal (DVE) + raw PV
                        # copies to SBUF (frees the PSUM banks quickly);
                        # broadcasts/normalizes defer into fill slots.
                        xs = drp.tile([98, QC], F32, name="xs")
                        nc.vector.tensor_scalar_max(xs[:], ps_d[0:98, :],
                                                    1e-30)
                        drf = drp.tile([98, QC], F32, name="drf")
                        nc.vector.reciprocal_approx_fast(drf[:], xs[:])
                        dr = drp.tile([98, QC], BF16, name="dr")
                        with nc.allow_low_precision(reason="denom bf16"):
                            nc.vector.tensor_copy(dr[:], drf[:])
                        raws = []
                        for ih, hp in enumerate(hps):
                            raw = ourp.tile([128, QC], BF16, tag="our",
                                            name="our")
                            with nc.allow_low_precision(reason="raw bf16"):
                                nc.vector.tensor_copy(raw[:], cur_ot[ih][:])
                            raws.append(raw)
                        for ih, hp in enumerate(hps):
                            fills.insert(ih, (lambda dr=dr, raw=raws[ih],
                                              j=j, hp=hp, ih=ih:
                                              norm_chain(dr, raw, j, hp, ih)))
                        if pair == 1 and j != TAIL_J:
                            # both pairs of chunk j normalized (after the
                            # two norm_chain fills above run): queue the
                            # output projection for chunk j.
                            for m in range(4):
                                for eo in range(2):
                                    fills.append(
                                        lambda j=j, m=m, eo=eo:
                                        outproj_group(j, m, eo))

                # drain leftover fills (the last unit's norm chains) while
                # psop is still open
                while fills:
                    fills.pop(0)()

            # ---- tail: last chunk's output projection ----
            with tc.tile_pool(name="tailp", bufs=4, space="PSUM") as tailp:
                for m in range(4):
                    for eo in range(2):
                        s0 = QC * TAIL_J + 128 * m
                        ps_o = tailp.tile([128, 512], F32, tag="tl",
                                          name="ps_t")
                        for hp in range(NHP):
                            nc.tensor.matmul(
                                ps_o[:],
                                lhsT=otn_map[(TAIL_J, hp)][:,
                                                           128 * m:
                                                           128 * (m + 1)],
                                rhs=wo_sb[hp][:, 512 * eo:512 * (eo + 1)],
                                start=(hp == 0), stop=(hp == NHP - 1))
                        osb = osbp.tile([128, 512], F32)
                        nc.vector.tensor_copy(osb[:], ps_o[:])
                        nc.sync.dma_start(
                            outp[s0:s0 + 128, 512 * eo:512 * (eo + 1)],
                            osb[:])
    nc.compile()
    return nc


def _get_nc():
    if "nc" not in _CACHE:
        _CACHE["nc"] = _build_nc()
    return _CACHE["nc"]


def _prep_core_inputs(x, w_qkv, b_qkv, w_out, b, hg):
    r0 = ELOC * hg
    wq = w_qkv[r0:r0 + ELOC, :]
    wk = w_qkv[D + r0:D + r0 + ELOC, :]
    wv = w_qkv[2 * D + r0:2 * D + r0 + ELOC, :]
    bq = b_qkv[r0:r0 + ELOC]
    bk = b_qkv[D + r0:D + r0 + ELOC]
    bv = b_qkv[2 * D + r0:2 * D + r0 + ELOC]

    bf = ml_dtypes.bfloat16
    bqk_arr = np.empty((128, 2, NHP), np.float32)
    bqk_arr[:, 0, :] = bq.reshape(NHP, 128).T
    bqk_arr[:, 1, :] = bk.reshape(NHP, 128).T
    return {
        "xT": np.ascontiguousarray(x[b].T).astype(bf),
        "wqT": np.ascontiguousarray(wq.T).astype(bf),
        "wkT": np.ascontiguousarray(wk.T).astype(bf),
        "wvT": np.ascontiguousarray(wv.T).astype(bf),
        "woT": np.ascontiguousarray(w_out[:, r0:r0 + ELOC].T).astype(bf),
        "bqk": bqk_arr,
        "bvb": np.tile(bv.astype(np.float32)[None, :], (128, 1)),
    }


def kernel(x, w_qkv, b_qkv, w_out, b_out, _trace=False, _trace_kwargs=None):
    x = np.asarray(x, np.float32)
    w_qkv = np.asarray(w_qkv, np.float32)
    b_qkv = np.asarray(b_qkv, np.float32)
    w_out = np.asarray(w_out, np.float32)
    b_out = np.asarray(b_out, np.float32)

    nc = _get_nc()
    in_maps = []
    for core in range(NCORES):
        b, hg = core // 2, core % 2
        in_maps.append(_prep_core_inputs(x, w_qkv, b_qkv, w_out, b, hg))

    kw = {}
    if _trace:
        kw.update(trace=True, **(_trace_kwargs or {}))
    import time
    res = None
    for attempt in range(4):
        try:
            res = bass_utils.run_bass_kernel_spmd(
                nc, in_maps, core_ids=list(range(NCORES)), **kw)
            break
        except Exception:
            if attempt == 3:
                raise
            # Transient axon/NRT device flake: reset the PJRT backend so the
            # retry starts from a clean client, like a fresh process would.
            try:
                import jax
                jax.clear_caches()
                import jax._src.xla_bridge as _xb
                _xb._clear_backends()
            except Exception:
                pass
            time.sleep(5.0 * (attempt + 1))

    out = np.empty((B, S, D), np.float32)
    for b in range(B):
        out[b] = res.results[2 * b]["outp"] + res.results[2 * b + 1]["outp"] \
            + b_out[None, :]
    if _trace:
        return out, res
    return out


# revision 26
# speedup vs baseline: 1.0016x; 1.0016x over previous
"""Causal multi-head attention (dense transformer block) on 8 Trainium2 cores.

Problem: x[4, 2048, 1024], 16 heads, head_dim 64, causal softmax attention
with QKV + output projections (torch Linear layout weights).

Sharding: 8 cores = 4 batches x 2 head-groups (8 heads each).  Each core
computes QKV projection for its 8 heads, attention, and its partial output
projection (row-parallel over w_out).  Host sums the two partials per batch
and adds b_out.

All device layouts are "transposed" so no on-device transposes are needed:
  - x is fed as xT [d, s]; Q^T/K^T are produced as [head_dim, s]
  - scores are computed as S^T [k, q]; softmax runs along partitions via
    ones-matmul column sums; O is accumulated directly as O^T [e_loc, q],
    which is exactly the lhsT the output projection needs.
Matmul inputs are bf16 (PSUM accumulation is fp32); everything else fp32.

Schedule (v2): the whole kernel is software-pipelined around the ACT
engine's exp stream, which is the irreducible resource (~151us):
  - head: V-projection starts inside the input-DMA window (kt-outer over 6
    PSUM banks), then only K(hp0,hp1) + Q(hp0,hp1;chunk3) run before
    attention begins (~37us to first exp instead of ~78us).
  - attention: flat iteration stream over units (j,pair), pair 0 units
    first.  Scores for iteration t+1 issue ahead of PV(t); the 4
    denominator ones-matmuls issue after both PV pairs so all four run
    tile-concurrently on disjoint 32-col groups (one ~213ns slot).
    Between the two half-periods a "fill" slot issues deferred work:
    V pass 2, remaining Q/K projection chains, output-projection groups,
    and softmax-denominator broadcast/normalize chains.
  - PV PSUM accumulators are copied to SBUF (raw, bf16) immediately at
    unit end so the PSUM banks recycle without waiting on the normalize
    chain; normalization multiplies happen later as deferred DVE work.
  - tail: only chunk 0's output projection, on a 4-buffer PSUM pool.
"""

import sys

sys.path.insert(0, "/opt/trn_rl_repo")

import numpy as np
import ml_dtypes

import concourse.bass as bass
import concourse.mybir as mybir
import concourse.tile as tile
from concourse import bacc
from concourse import bass_utils
from concourse import library_config
from concourse.masks import make_upper_triangular

F32 = mybir.dt.float32
BF16 = mybir.dt.bfloat16
EXP = mybir.ActivationFunctionType.Exp

B, S, D = 4, 2048, 1024
HTOT, HD = 16, 64
NCORES = 8
HLOC = HTOT // 2          # heads per core
ELOC = HLOC * HD          # 512 local embedding width
NHP = HLOC // 2           # 4 head pairs
QC = 512                  # q-chunk width
NQC = S // QC             # 4
NKT = S // 128            # 16 k tiles over sequence
NDT = D // 128            # 8 k tiles over model dim
SCALE = 1.0 / float(np.sqrt(HD))

_CACHE = {}


def _build_nc():
    nc = bacc.Bacc("TRN2", target_bir_lowering=False, debug=False)

    xT = nc.dram_tensor("xT", [D, S], BF16, kind="ExternalInput")
    wqT = nc.dram_tensor("wqT", [D, ELOC], BF16, kind="ExternalInput")
    wkT = nc.dram_tensor("wkT", [D, ELOC], BF16, kind="ExternalInput")
    wvT = nc.dram_tensor("wvT", [D, ELOC], BF16, kind="ExternalInput")
    woT = nc.dram_tensor("woT", [ELOC, D], BF16, kind="ExternalInput")
    bqk = nc.dram_tensor("bqk", [128, 2, NHP], F32, kind="ExternalInput")
    bvb = nc.dram_tensor("bvb", [128, ELOC], F32, kind="ExternalInput")
    outp = nc.dram_tensor("outp", [S, D], F32, kind="ExternalOutput")

    with tile.TileContext(nc) as tc:
        with tc.tile_pool(name="const", bufs=1) as constp, \
             tc.tile_pool(name="wpool", bufs=1) as wp, \
             tc.tile_pool(name="qkv", bufs=1) as qkvp, \
             tc.tile_pool(name="xt", bufs=1) as xtp, \
             tc.tile_pool(name="pt", bufs=8) as ptp, \
             tc.tile_pool(name="otn", bufs=16) as otnp, \
             tc.tile_pool(name="our", bufs=12) as ourp, \
             tc.tile_pool(name="dr", bufs=4) as drp, \
             tc.tile_pool(name="osb", bufs=6) as osbp:

            # ---- constants ----
            # wsrc feeds the PE warm-up matmuls; memset on DVE so the PE can
            # start at ~0 instead of queueing behind gpsimd const work.
            wsrc = constp.tile([128, ELOC], BF16, name="wsrc")
            nc.vector.memset(wsrc[:], 0.0)
            trimask = constp.tile([128, 128], BF16, name="trimask")
            make_upper_triangular(nc, trimask[:], val=1.0, diag=True)
            ones_a = constp.tile([128, 1], BF16, name="ones_a")
            nc.gpsimd.memset(ones_a[:], 1.0)
            ones_b = constp.tile([128, 2], BF16, name="ones_b")
            nc.gpsimd.memset(ones_b[:], 0.0)
            nc.gpsimd.memset(ones_b[:, 0:1], 1.0)
            bc_x = constp.tile([98, 128], BF16, name="bc_x")
            nc.gpsimd.memset(bc_x[:], 0.0)
            nc.gpsimd.memset(bc_x[0:1, 0:64], 1.0)
            nc.gpsimd.memset(bc_x[32:33, 64:128], 1.0)
            bc_y = constp.tile([98, 128], BF16, name="bc_y")
            nc.gpsimd.memset(bc_y[:], 0.0)
            nc.gpsimd.memset(bc_y[64:65, 0:64], 1.0)
            nc.gpsimd.memset(bc_y[96:97, 64:128], 1.0)

            bqk_sb = constp.tile([128, 2, NHP], F32, name="bqk_sb")
            nc.sync.dma_start(bqk_sb[:], bqk[:])
            bvb_sb = constp.tile([128, ELOC], F32, name="bvb_sb")
            nc.sync.dma_start(bvb_sb[:], bvb[:])

            # ---- input DMAs: V-projection inputs first, interleaved so
            # the kt-outer V pass can start as soon as (wv0, xt0) land.
            # wq/wk/wo issue from other sequencers to parallelize DGE setup.
            wv_sb, xts = [], []
            for kt in range(NDT):
                t = wp.tile([128, ELOC], BF16, name=f"wv{kt}")
                nc.sync.dma_start(t[:], wvT[128 * kt:128 * (kt + 1), :])
                wv_sb.append(t)
                t = xtp.tile([128, S], BF16, name=f"xt{kt}")
                nc.sync.dma_start(t[:], xT[128 * kt:128 * (kt + 1), :])
                xts.append(t)
            wq_sb, wk_sb = [], []
            for kt in range(NDT):
                for lst, srct, nm in ((wq_sb, wqT, "wq"), (wk_sb, wkT, "wk")):
                    t = wp.tile([128, ELOC], BF16, name=f"{nm}{kt}")
                    nc.scalar.dma_start(t[:], srct[128 * kt:128 * (kt + 1), :])
                    lst.append(t)
            wo_sb = []
            for hp in range(NHP):
                t = wp.tile([128, D], BF16, name=f"wo{hp}")
                nc.scalar.dma_start(t[:], woT[128 * hp:128 * (hp + 1), :])
                wo_sb.append(t)

            # ---- QKV destination tiles ----
            QT, KT = [], []
            for hp in range(NHP):
                QT.append(qkvp.tile([128, S], BF16, name=f"qt{hp}"))
                KT.append(qkvp.tile([128, S], BF16, name=f"kt{hp}"))
            V = [qkvp.tile([128, ELOC], BF16, name=f"v{st}") for st in range(NKT)]

            def v_chain(pool, tag, st):
                ps = pool.tile([128, ELOC], F32, tag=tag, name="ps_f")
                for kt in range(NDT):
                    nc.tensor.matmul(
                        ps[:],
                        lhsT=xts[kt][:, 128 * st:128 * (st + 1)],
                        rhs=wv_sb[kt][:],
                        start=(kt == 0), stop=(kt == NDT - 1))
                nc.vector.tensor_add(V[st][:], ps[:], bvb_sb[:])

            def qk_chain(pool, tag, qk, hp, c):
                dst = (QT if qk == 0 else KT)[hp]
                wsb = wq_sb if qk == 0 else wk_sb
                ps = pool.tile([128, QC], F32, tag=tag, name="ps_f")
                for kt in range(NDT):
                    nc.tensor.matmul(
                        ps[:],
                        lhsT=wsb[kt][:, 128 * hp:128 * (hp + 1)],
                        rhs=xts[kt][:, QC * c:QC * (c + 1)],
                        start=(kt == 0), stop=(kt == NDT - 1))
                nc.vector.tensor_scalar_add(
                    dst[:, QC * c:QC * (c + 1)], ps[:],
                    bqk_sb[:, qk, hp:hp + 1])

            # ---- head: V pass 1 (st 0..5 kt-outer inside the DMA window,
            # then st 6,7), K(0), K(1) all chunks, Q(0,c3), Q(1,c3). ----
            with tc.tile_pool(name="psq", bufs=8, space="PSUM") as psq:
                # PE warm-up at t~0: keeps the HAM activity monitor busy so
                # the first real matmuls run at 2.4 GHz.  Depends only on
                # the DVE wsrc memset, not the DMAs or gpsimd consts.
                wups = psq.tile([128, ELOC], F32, tag="psq", name="wups")
                for r in range(20):
                    nc.tensor.matmul(wups[:], lhsT=wsrc[:, 0:128],
                                     rhs=wsrc[:],
                                     start=(r == 0), stop=(r == 19))
                # mega kt-loop using all 8 PSUM banks: V st0-3 and K(0)'s
                # four chunks accumulate together, so the DMA window is
                # filled with ~8 matmuls per arriving (wv,xt,wk) tile set.
                psv = [psq.tile([128, ELOC], F32, tag="psq", name=f"psv{st}")
                       for st in range(4)]
                psk = [psq.tile([128, QC], F32, tag="psq", name=f"psk{c}")
                       for c in range(NQC)]
                for kt in range(NDT - 1):
                    for st in range(4):
                        nc.tensor.matmul(
                            psv[st][:],
                            lhsT=xts[kt][:, 128 * st:128 * (st + 1)],
                            rhs=wv_sb[kt][:],
                            start=(kt == 0), stop=False)
                    for c in range(NQC):
                        nc.tensor.matmul(
                            psk[c][:],
                            lhsT=wk_sb[kt][:, 0:128],
                            rhs=xts[kt][:, QC * c:QC * (c + 1)],
                            start=(kt == 0), stop=False)
                # peel kt=7 so each PSUM drain (DVE add) pipelines behind
                # the next accumulator's matmul instead of serializing at
                # the end (which idled the PE enough to re-throttle HAM).
                for st in range(4):
                    nc.tensor.matmul(
                        psv[st][:],
                        lhsT=xts[NDT - 1][:, 128 * st:128 * (st + 1)],
                        rhs=wv_sb[NDT - 1][:],
                        start=False, stop=True)
                    nc.vector.tensor_add(V[st][:], psv[st][:], bvb_sb[:])
                for c in range(NQC):
                    nc.tensor.matmul(
                        psk[c][:],
                        lhsT=wk_sb[NDT - 1][:, 0:128],
                        rhs=xts[NDT - 1][:, QC * c:QC * (c + 1)],
                        start=False, stop=True)
                    nc.vector.tensor_scalar_add(
                        KT[0][:, QC * c:QC * (c + 1)], psk[c][:],
                        bqk_sb[:, 1, 0:1])
                for st in (4, 5, 6, 7):
                    v_chain(psq, "psq", st)
                # K(1) kt-outer over 4 chunk-banks (shared stationary wk
                # slice across the inner matmuls).
                psl = [psq.tile([128, QC], F32, tag="psq", name=f"psl{c}")
                       for c in range(NQC)]
                for kt in range(NDT - 1):
                    for c in range(NQC):
                        nc.tensor.matmul(
                            psl[c][:],
                            lhsT=wk_sb[kt][:, 128:256],
                            rhs=xts[kt][:, QC * c:QC * (c + 1)],
                            start=(kt == 0), stop=False)
                for c in range(NQC):
                    nc.tensor.matmul(
                        psl[c][:],
                        lhsT=wk_sb[NDT - 1][:, 128:256],
                        rhs=xts[NDT - 1][:, QC * c:QC * (c + 1)],
                        start=False, stop=True)
                    nc.vector.tensor_scalar_add(
                        KT[1][:, QC * c:QC * (c + 1)], psl[c][:],
                        bqk_sb[:, 1, 1:2])
                qk_chain(psq, "psq", 0, 0, 3)
                qk_chain(psq, "psq", 0, 1, 3)

            # ---- attention: flat pipelined stream over units ----
            # units: pair 0 first so K/Q for hps 2,3 and all output
            # projections can be deferred into fill slots.  Within pair 1,
            # small chunks run first so each chunk's output projection has
            # enough later fill slots; the big (3,1) unit is last and its
            # output projection runs in the tail.
            units = [(3, 0), (2, 0), (1, 0), (0, 0),
                     (1, 1), (0, 1), (2, 1), (3, 1)]
            iters = []
            for u, (j, pair) in enumerate(units):
                nkt = 4 * j + 4
                for i in range(nkt):
                    iters.append((u, j, pair, i, nkt))
            NIT = len(iters)

            tri3 = trimask[:][:, None, :].broadcast_to([128, 2, 128])

            # deferred-work queue; one item issued per iteration between the
            # two half-periods.  Items are closures issuing ~0.2-1.8us of PE
            # work (plus DVE followups).
            fills = []
            # V pass 2 then the Q/K chains needed before their deadlines:
            # Q(0/1,c2) by unit (2,0); K2/K3 and Q(2/3,c3) by unit (3,1);
            # Q(0/1,c1) by (1,0); Q(0/1,c0) by (0,0); Q(2/3,c2..c0) by the
            # matching pair-1 units.  Pops: item k at global iteration k.
            for st in range(8, NKT):
                fills.append(lambda st=st: v_chain(psop, "ps", st))
            for hp in (0, 1):
                fills.append(lambda hp=hp: qk_chain(psop, "ps", 0, hp, 2))
            for c in range(NQC):
                fills.append(lambda c=c: qk_chain(psop, "ps", 1, 2, c))
            for c in range(NQC):
                fills.append(lambda c=c: qk_chain(psop, "ps", 1, 3, c))
            for hp in (0, 1):
                fills.append(lambda hp=hp: qk_chain(psop, "ps", 0, hp, 1))
            for hp in (0, 1):
                fills.append(lambda hp=hp: qk_chain(psop, "ps", 0, hp, 0))
            fills.append(lambda: qk_chain(psop, "ps", 0, 2, 3))
            fills.append(lambda: qk_chain(psop, "ps", 0, 3, 3))
            for c in (2, 1, 0):
                fills.append(lambda c=c: qk_chain(psop, "ps", 0, 2, c))
                fills.append(lambda c=c: qk_chain(psop, "ps", 0, 3, c))

            otn_map = {}   # (j, hp) -> normalized O^T tile (bf16, SBUF)

            def outproj_group(j, m, eo):
                s0 = QC * j + 128 * m
                ps_o = psop.tile([128, 512], F32, tag="ps", name="ps_f")
                for hp in range(NHP):
                    nc.tensor.matmul(
                        ps_o[:],
                        lhsT=otn_map[(j, hp)][:, 128 * m:128 * (m + 1)],
                        rhs=wo_sb[hp][:, 512 * eo:512 * (eo + 1)],
                        start=(hp == 0), stop=(hp == NHP - 1))
                osb = osbp.tile([128, 512], F32)
                nc.vector.tensor_copy(osb[:], ps_o[:])
                nc.sync.dma_start(
                    outp[s0:s0 + 128, 512 * eo:512 * (eo + 1)], osb[:])

            def norm_chain(dr, raw, j, hp, ih):
                # broadcast the two denominator-reciprocal rows for this hp
                # to 128 partitions via PE, then normalize the raw PV copy.
                bc = bc_x if ih == 0 else bc_y
                ps_b = psop.tile([128, QC], F32, tag="ps", name="ps_f")
                nc.tensor.matmul(ps_b[:], lhsT=bc[:], rhs=dr[:],
                                 start=True, stop=True)
                dbc = drp.tile([128, QC], BF16, name="dbc")
                nc.vector.tensor_copy(dbc[:], ps_b[:])
                otn = otnp.tile([128, QC], BF16, tag="otn", name="otn")
                nc.vector.tensor_mul(otn[:], raw[:], dbc[:])
                otn_map[(j, hp)] = otn

            TAIL_J = units[-1][0]

            with tc.tile_pool(name="pss", bufs=2, space="PSUM") as pss, \
                 tc.tile_pool(name="psov", bufs=2, space="PSUM") as psov, \
                 tc.tile_pool(name="psd", bufs=1, space="PSUM") as psd, \
                 tc.tile_pool(name="psop", bufs=1, space="PSUM") as psop:

                def issue_scores(t, ih):
                    u, j, pair, i, nkt = iters[t]
                    hp = 2 * pair + ih
                    w = 128 * (i - 4 * j) if i >= 4 * j else 0
                    ps_s = pss.tile([128, 2, QC], F32, tag="pss",
                                    name="ps_s")
                    for h2 in range(2):
                        nc.tensor.matmul(
                            ps_s[:, h2, w:QC],
                            lhsT=KT[hp][64 * h2:64 * (h2 + 1),
                                        128 * i:128 * (i + 1)],
                            rhs=QT[hp][64 * h2:64 * (h2 + 1),
                                       QC * j + w:QC * (j + 1)],
                            start=True, stop=True)
                    return ps_s

                # prologue: scores for iteration 0
                sc_next = [issue_scores(0, 0), issue_scores(0, 1)]
                cur_ot = None   # [ps_ot_a, ps_ot_b] accumulators of the unit
                ps_d = None

                for t in range(NIT):
                    u, j, pair, i, nkt = iters[t]
                    hps = (2 * pair, 2 * pair + 1)
                    w = 128 * (i - 4 * j) if i >= 4 * j else 0
                    last = (i == nkt - 1)
                    ps_pair = sc_next
                    sc_next = [None, None]

                    if i == 0:
                        cur_ot = [psov.tile([128, QC], F32, tag="psov",
                                            name=f"ps_ot{hp}")
                                  for hp in hps]
                        ps_d = psd.tile([128, QC], F32, tag="psd",
                                        name="ps_d")

                    # exp for both hps (ACT stream; masks on DVE)
                    pts = []
                    for ih, hp in enumerate(hps):
                        pt = ptp.tile([128, 2, QC], BF16, tag="pt",
                                      name="pt")
                        nc.scalar.activation(pt[:, :, w:QC],
                                             ps_pair[ih][:, :, w:QC],
                                             EXP, scale=SCALE)
                        if i >= 4 * j:
                            nc.vector.tensor_mul(
                                pt[:, :, w:w + 128],
                                pt[:, :, w:w + 128], tri3[:, :, :])
                        pts.append(pt)

                    def issue_pv(ih, hp):
                        for h2 in range(2):
                            nc.tensor.matmul(
                                cur_ot[ih][64 * h2:64 * (h2 + 1), w:QC],
                                lhsT=V[i][:, 64 * (2 * hp + h2):
                                          64 * (2 * hp + h2 + 1)],
                                rhs=pts[ih][:, h2, w:QC],
                                start=(i == 0), stop=last,
                                tile_position=(0, 64 * h2))

                    # PE order: lead with the instruction whose gate opens
                    # first (PV(t,a) on exp(t,a); scores lead on masked
                    # boundary tiles since PV also waits for the DVE mask),
                    # then keep the two score pairs adjacent — same-kind
                    # matmuls stream back-to-back with no weight-load
                    # exposure, unlike kind-switches (~+100ns each).
                    bdry = i >= 4 * j
                    if bdry:
                        if t + 1 < NIT:
                            sc_next[0] = issue_scores(t + 1, 0)
                            sc_next[1] = issue_scores(t + 1, 1)
                        issue_pv(0, hps[0])
                        issue_pv(1, hps[1])
                    else:
                        issue_pv(0, hps[0])
                        if t + 1 < NIT:
                            sc_next[0] = issue_scores(t + 1, 0)
                            sc_next[1] = issue_scores(t + 1, 1)
                        issue_pv(1, hps[1])

                    for ih in range(2):
                        nc.tensor.matmul(
                            ps_d[64 * ih:64 * ih + 1, w:QC],
                            lhsT=ones_a[:], rhs=pts[ih][:, 0, w:QC],
                            start=(i == 0), stop=last,
                            tile_position=(0, 64 * ih))
                        nc.tensor.matmul(
                            ps_d[64 * ih + 32:64 * ih + 34, w:QC],
                            lhsT=ones_b[:], rhs=pts[ih][:, 1, w:QC],
                            start=(i == 0), stop=last,
                            tile_position=(0, 64 * ih + 32))

                    if fills:
                        fills.pop(0)()

                    if last:
                        # unit end: denominator reciprocal (DVE) + raw PV
                        # copies to SBUF (frees the PSUM banks quickly);
                        # broadcasts/normalizes defer into fill slots.
                        xs = drp.tile([98, QC], F32, name="xs")
                        nc.vector.tensor_scalar_max(xs[:], ps_d[0:98, :],
                                                    1e-30)
                        drf = drp.tile([98, QC], F32, name="drf")
                        nc.vector.reciprocal_approx_fast(drf[:], xs[:])
                        dr = drp.tile([98, QC], BF16, name="dr")
                        with nc.allow_low_precision(reason="denom bf16"):
                            nc.vector.tensor_copy(dr[:], drf[:])
                        raws = []
                        for ih, hp in enumerate(hps):
                            raw = ourp.tile([128, QC], BF16, tag="our",
                                            name="our")
                            with nc.allow_low_precision(reason="raw bf16"):
                                nc.vector.tensor_copy(raw[:], cur_ot[ih][:])
                            raws.append(raw)
                        for ih, hp in enumerate(hps):
                            fills.insert(ih, (lambda dr=dr, raw=raws[ih],
                                              j=j, hp=hp, ih=ih:
                                              norm_chain(dr, raw, j, hp, ih)))
                        if pair == 1 and j != TAIL_J:
                            # both pairs of chunk j normalized (after the
                            # two norm_chain fills above run): queue the
                            # output projection for chunk j.
                            for m in range(4):
                                for eo in range(2):
                                    fills.append(
                                        lambda j=j, m=m, eo=eo:
                                        outproj_group(j, m, eo))

                # drain leftover fills (the last unit's norm chains) while
                # psop is still open
                while fills:
                    fills.pop(0)()

            # ---- tail: last chunk's output projection ----
            with tc.tile_pool(name="tailp", bufs=4, space="PSUM") as tailp:
                for m in range(4):
                    for eo in range(2):
                        s0 = QC * TAIL_J + 128 * m
                        ps_o = tailp.tile([128, 512], F32, tag="tl",
                                          name="ps_t")
                        for hp in range(NHP):
                            nc.tensor.matmul(
                                ps_o[:],
                                lhsT=otn_map[(TAIL_J, hp)][:,
                                                           128 * m:
                                                           128 * (m + 1)],
                                rhs=wo_sb[hp][:, 512 * eo:512 * (eo + 1)],
                                start=(hp == 0), stop=(hp == NHP - 1))
                        osb = osbp.tile([128, 512], F32)
                        nc.vector.tensor_copy(osb[:], ps_o[:])
                        nc.sync.dma_start(
                            outp[s0:s0 + 128, 512 * eo:512 * (eo + 1)],
                            osb[:])
    nc.compile()
    return nc


def _get_nc():
    if "nc" not in _CACHE:
        _CACHE["nc"] = _build_nc()
    return _CACHE["nc"]


def _prep_core_inputs(x, w_qkv, b_qkv, w_out, b, hg):
    r0 = ELOC * hg
    wq = w_qkv[r0:r0 + ELOC, :]
    wk = w_qkv[D + r0:D + r0 + ELOC, :]
    wv = w_qkv[2 * D + r0:2 * D + r0 + ELOC, :]
    bq = b_qkv[r0:r0 + ELOC]
    bk = b_qkv[D + r0:D + r0 + ELOC]
    bv = b_qkv[2 * D + r0:2 * D + r0 + ELOC]

    bf = ml_dtypes.bfloat16
    bqk_arr = np.empty((128, 2, NHP), np.float32)
    bqk_arr[:, 0, :] = bq.reshape(NHP, 128).T
    bqk_arr[:, 1, :] = bk.reshape(NHP, 128).T
    return {
        "xT": np.ascontiguousarray(x[b].T).astype(bf),
        "wqT": np.ascontiguousarray(wq.T).astype(bf),
        "wkT": np.ascontiguousarray(wk.T).astype(bf),
        "wvT": np.ascontiguousarray(wv.T).astype(bf),
        "woT": np.ascontiguousarray(w_out[:, r0:r0 + ELOC].T).astype(bf),
        "bqk": bqk_arr,
        "bvb": np.tile(bv.astype(np.float32)[None, :], (128, 1)),
    }


def kernel(x, w_qkv, b_qkv, w_out, b_out, _trace=False, _trace_kwargs=None):
    x = np.asarray(x, np.float32)
    w_qkv = np.asarray(w_qkv, np.float32)
    b_qkv = np.asarray(b_qkv, np.float32)
    w_out = np.asarray(w_out, np.float32)
    b_out = np.asarray(b_out, np.float32)

    nc = _get_nc()
    in_maps = []
    for core in range(NCORES):
        b, hg = core // 2, core % 2
        in_maps.append(_prep_core_inputs(x, w_qkv, b_qkv, w_out, b, hg))

    kw = {}
    if _trace:
        kw.update(trace=True, **(_trace_kwargs or {}))
    import time
    res = None
    for attempt in range(4):
        try:
            res = bass_utils.run_bass_kernel_spmd(
                nc, in_maps, core_ids=list(range(NCORES)), **kw)
            break
        except Exception:
            if attempt == 3:
                raise
            # Transient axon/NRT device flake: reset the PJRT backend so the
            # retry starts from a clean client, like a fresh process would.
            try:
                import jax
                jax.clear_caches()
                import jax._src.xla_bridge as _xb
                _xb._clear_backends()
            except Exception:
                pass
            time.sleep(5.0 * (attempt + 1))

    out = np.empty((B, S, D), np.float32)
    for b in range(B):
        out[b] = res.results[2 * b]["outp"] + res.results[2 * b + 1]["outp"] \
            + b_out[None, :]
    if _trace:
        return out, res
    return out
